# revision 23
# baseline (speedup 1.0000x reference)
"""Trainium2 Bass kernel for fused attention prefill (nn_Attn_50740743635107).

Reference computation (fp32):
  qkv = x @ W_qkv.T ; split q,k,v ; interleaved RoPE on q,k ;
  scores = q k^T / sqrt(dh) with causal+valid_k mask ; softmax ;
  ctx = attn @ v ; out = ctx @ W_out.T

Shapes: B=4, S=1024, D=2048, H=16, DH=128.

Sharding: 8 cores = 4 batches x 2 head-groups (8 heads each).
Each core computes a partial out^T for its (batch, head-group);
the host sums the two head-group partials per batch and transposes.

v2 design (vs fp32r v1):
- All operands bf16 except the q/k projection which runs fp8e4m3 with
  DoubleRow perf mode (2 k-tiles per matmul). Host bakes power-of-2
  scales (x*8, Wqk*64); the combined descale folds into the softmax
  exp's free scale operand.
- Host pre-arranges every DRAM tensor into its exact SBUF layout so all
  DMAs are contiguous.
- RoPE in bf16 on DVE/GpSimd (4 tensor_tensor ops per 128-row tile, 2x
  DVE mode); q/k rows pre-permuted so even/odd pairs become contiguous
  halves.
- Causal mask applied multiplicatively AFTER exp (0/1 bf16 tri tile) so
  exp reads score PSUM directly; ragged seq_len masking stays as a
  per-partition additive bias inside the exp activation.
- Softmax reciprocal via ACT ln + exp(-x) (both functions live in the
  natural_log_exp table set) instead of the slow DVE iterative divide.
- Denominators via an all-ones stationary matmul accumulated alongside
  ctx, giving a partition-broadcast denominator for the normalize.
"""

from contextlib import ExitStack

import numpy as np
import ml_dtypes

import concourse.bass as bass
from concourse import bacc
from concourse import hw_specs
import concourse.mybir as mybir
import concourse.tile as tile
from concourse.bass_utils import run_bass_kernel_spmd

B, S, D, H = 4, 1024, 2048, 16
DH = 128           # head dim
HPC = 8            # heads per core
DC = HPC * DH      # 1024: d-range per core
P = 128
THETA = 10000.0
NEG = -60.0
F32 = mybir.dt.float32
BF16 = mybir.dt.bfloat16
FP8 = mybir.dt.float8e4
MULT = mybir.AluOpType.mult
ADD = mybir.AluOpType.add
SUB = mybir.AluOpType.subtract
EXP = mybir.ActivationFunctionType.Exp
LN = mybir.ActivationFunctionType.Ln
COPY = mybir.ActivationFunctionType.Copy
DR = mybir.MatmulPerfMode.DoubleRow

FP8_QK = True      # fp8 DoubleRow q/k projection (bf16 fallback if False)
SX, SW = 8.0, 64.0
_F = (SX * SW) if FP8_QK else 1.0
SCALE = float(1.0 / (_F * _F * np.sqrt(DH)))   # softmax exp input scale

# score tiles per head: ALLOWED[sq_half] = sk tiles; diagonal tiles are
# masked via the 0/1 tri tile, (t>=4, sh=1) also get the seq-len bias.
ALLOWED = {0: [0, 1, 2, 3], 1: [0, 1, 2, 3, 4, 5, 6, 7]}
PARTIAL = {(t, 0) for t in range(4)} | {(t, 1) for t in range(4, 8)}


def build_nc(reps=1):
    nc = bacc.Bacc()
    qk_dt = FP8 if FP8_QK else BF16
    x8_d = nc.dram_tensor("x8", [P, 16, S], qk_dt, kind="ExternalInput")
    xb_d = nc.dram_tensor("xb", [P, 16, S], BF16, kind="ExternalInput")
    w8_d = nc.dram_tensor("w8", [P, 16, 16, P], qk_dt, kind="ExternalInput")
    wv_d = nc.dram_tensor("wv", [P, 2, 16, 512], BF16, kind="ExternalInput")
    wo_d = nc.dram_tensor("wo", [P, 8, D], BF16, kind="ExternalInput")
    cs_d = nc.dram_tensor("cs", [P, 2, S], BF16, kind="ExternalInput")
    tri_d = nc.dram_tensor("tri", [P, P], BF16, kind="ExternalInput")
    bias_d = nc.dram_tensor("bias", [P, 4], F32, kind="ExternalInput")
    ones_d = nc.dram_tensor("ones", [P, P], BF16, kind="ExternalInput")
    ones8_d = nc.dram_tensor("ones8", [P, 2, P], FP8, kind="ExternalInput")
    outT_d = nc.dram_tensor("outT", [P, 16, S], BF16, kind="ExternalOutput")

    with tile.TileContext(nc) as tc:
      for rep in range(reps):
        with ExitStack() as es:
            pool = lambda **kw: es.enter_context(tc.tile_pool(**kw))
            qktp = pool(name="qkt", bufs=1)    # [128,16,1024] bf16 32K/p
            vsbp = pool(name="vsb", bufs=1)    # [128,8,1024] bf16 16K/p
            v8p = pool(name="v8", bufs=1)      # [128,8,1024] fp8 8K/p
            xbp = pool(name="xb", bufs=1)      # bf16 32K/p
            wvp = pool(name="wv", bufs=2)      # [128,16,512] 16K/p bf16
            cstbp = pool(name="cstb", bufs=1)  # tri/bias/ones consts
            gtp = pool(name="gt", bufs=1)      # dma-gate scratch
            psp = pool(name="ps", bufs=4, space=bass.MemorySpace.PSUM)
            qkT = qktp.tile([P, 16, S], BF16, tag="qkt")
            vsb = vsbp.tile([P, 8, DC], BF16, tag="vsb")
            v8 = v8p.tile([P, 8, DC], FP8, tag="v8")
            xbt = xbp.tile([P, 16, S], BF16, tag="xb")
            tri_t = cstbp.tile([P, P], BF16, tag="tri")
            nc.gpsimd.dma_start(tri_t[:], tri_d[:])
            bias_t = cstbp.tile([P, 4], F32, tag="bias")
            nc.gpsimd.dma_start(bias_t[:], bias_d[:])
            ones_t = cstbp.tile([P, P], BF16, tag="ones")
            nc.gpsimd.dma_start(ones_t[:], ones_d[:])
            ones8_t = cstbp.tile([P, 2, P], FP8, tag="ones8")
            nc.gpsimd.dma_start(ones8_t[:], ones8_d[:])

            # ================= phase A1: q/k projection + RoPE ===============
            with ExitStack() as esA:
                poolA = lambda **kw: esA.enter_context(tc.tile_pool(**kw))
                x8p = poolA(name="x8", bufs=1)     # fp8 16K/p (bf16 32K)
                wqkp = poolA(name="wqk", bufs=1)   # [128,16,16,128] 32K/p
                cstp = poolA(name="cst", bufs=1)   # cos/sin 4K/p
                ropep = poolA(name="rope", bufs=2) # [128,1024] bf16 2K/p
                pswp = poolA(name="psw", bufs=1, space=bass.MemorySpace.PSUM)
                x8t = x8p.tile([P, 16, S], qk_dt, tag="x8")
                w8t = wqkp.tile([P, 16, 16, P], qk_dt, tag="wqk")
                cs_t = cstp.tile([P, 2, S], BF16, tag="cs")
                with tc.high_priority():
                    # cs first (small) so PE warmup matmuls can start, then
                    # the tensors gating the first q/k m-tiles. x on sync,
                    # weights on the scalar HWDGE queue, in parallel.
                    nc.sync.dma_start(cs_t[:], cs_d[:])
                    nc.sync.dma_start(x8t[:], x8_d[:])
                    for quarter in range(4):
                        eng = nc.scalar if quarter % 2 == 0 else nc.sync
                        eng.dma_start(
                            w8t[:, 4 * quarter : 4 * (quarter + 1)],
                            w8_d[:, 4 * quarter : 4 * (quarter + 1)],
                        )
                    # PE warmup: ~14us of throwaway matmuls so the HAM clock
                    # gate reaches 8/8 and the x/w DMA wait is masked; gated
                    # only on the small cs transfer. Output is never read.
                    warm_ps = pswp.tile([P, 512], F32, tag="psw")
                    for w in range(56):
                        nc.tensor.matmul(
                            warm_ps[:], cs_t[:, 0, 0:P], cs_t[:, 0, 0:512],
                            start=True, stop=True,
                        )

                # ---- q/k projection (m-tile mt: 0..7 = q heads, 8..15 = k)
                for mt in range(16):
                    wqk = w8t[:, mt]
                    ps0 = psp.tile([P, 512], F32, tag="ps", name=f"qk{mt}_0")
                    ps1 = psp.tile([P, 512], F32, tag="ps", name=f"qk{mt}_1")
                    if FP8_QK:
                        for kk in range(8):
                            lw = wqk[:, 2 * kk : 2 * kk + 2, :]
                            nc.tensor.matmul(
                                ps0[:], lw, x8t[:, 2 * kk : 2 * kk + 2, 0:512],
                                start=(kk == 0), stop=(kk == 7), perf_mode=DR,
                            )
                            nc.tensor.matmul(
                                ps1[:], lw, x8t[:, 2 * kk : 2 * kk + 2, 512:1024],
                                start=(kk == 0), stop=(kk == 7), perf_mode=DR,
                            )
                    else:
                        for kt in range(16):
                            nc.tensor.matmul(
                                ps0[:], wqk[:, kt, :], x8t[:, kt, 0:512],
                                start=(kt == 0), stop=(kt == 15),
                            )
                            nc.tensor.matmul(
                                ps1[:], wqk[:, kt, :], x8t[:, kt, 512:1024],
                                start=(kt == 0), stop=(kt == 15),
                            )
                    col = qkT[:, mt, :]
                    nc.scalar.activation(col[:, 0:512], ps0[:], COPY)
                    nc.scalar.activation(col[:, 512:1024], ps1[:], COPY)
                    # ---- RoPE in place on qkT[:, mt, :].
                    # rows 0..63 = even dh (xe), 64..127 = odd dh (xo):
                    #   new_e = xe*cos - xo*sin ; new_o = xo*cos + xe*sin
                    # (both TT inputs must share a base partition; only the
                    # output AP may be partition-shifted)
                    tmp = ropep.tile([P, S], BF16, tag="rope")
                    nc.vector.tensor_tensor(
                        tmp[0:64, :], col[64:128, :], cs_t[64:128, 1, :], op=MULT
                    )
                    nc.vector.tensor_tensor(
                        tmp[64:128, :], col[0:64, :], cs_t[0:64, 1, :], op=MULT
                    )
                    nc.vector.tensor_tensor(col[:], col[:], cs_t[:, 0, :], op=MULT)
                    nc.vector.tensor_tensor(
                        col[0:64, :], col[0:64, :], tmp[0:64, :], op=SUB
                    )
                    nc.vector.tensor_tensor(
                        col[64:128, :], col[64:128, :], tmp[64:128, :], op=ADD
                    )

            # Pools for attention + out-projection, created only after the
            # A1 pools have released their SBUF (the allocator reserves pool
            # space in creation order for the pool's scope lifetime).
            ctxp = pool(name="ctx", bufs=1)    # [128,8,1024] bf16 16K/p
            exps = pool(name="ex", bufs=10)    # [128,512] bf16 1K/p
            rcp = pool(name="rc", bufs=3)      # [128,512] f32 2K/p
            cup = pool(name="cu", bufs=3)      # [128,512] bf16 1K/p
            wop = pool(name="wo", bufs=1)      # [128,8,2048] bf16 32K/p
            otp = pool(name="ot", bufs=3)      # [128,512] bf16 1K/p
            pscp = pool(name="psc", bufs=2, space=bass.MemorySpace.PSUM)
            psdp = pool(name="psd", bufs=2, space=bass.MemorySpace.PSUM)

            # Late bulk DMAs (xb/wv/wo), gated on mt0 being roped so they do
            # not steal HBM bandwidth from the startup-critical x8/w8 loads.
            gate = gtp.tile([1, 2], BF16, tag="gt")
            nc.gpsimd.tensor_copy(gate[:], qkT[0:1, 0, 0:2])
            for half in range(2):
                nc.gpsimd.dma_start(
                    xbt[:, 8 * half : 8 * (half + 1), :],
                    xb_d[:, 8 * half : 8 * (half + 1), :],
                )
            wvts = []
            for nhp in range(2):
                wvt = wvp.tile([P, 16, 512], BF16, tag="wv", name=f"wv{nhp}")
                nc.gpsimd.dma_start(wvt[:], wv_d[:, nhp])
                wvts.append(wvt)
            ctxT = ctxp.tile([P, 8, S], BF16, tag="ctx")
            wo_t = wop.tile([P, 8, D], BF16, tag="wo")
            nc.gpsimd.dma_start(wo_t[:], wo_d[:])

            # ---- v projection block: v[s, j] = sum_d xb[d, s] * Wv[j, d]
            def emit_v_block(nhp, st):
                psv = psp.tile([P, 512], F32, tag="ps", name=f"v{nhp}_{st}")
                for kt in range(16):
                    nc.tensor.matmul(
                        psv[:],
                        xbt[:, kt, P * st : P * (st + 1)],
                        wvts[nhp][:, kt, :],
                        start=(kt == 0),
                        stop=(kt == 15),
                    )
                nc.scalar.copy(vsb[:, st, 512 * nhp : 512 * (nhp + 1)], psv[:])
                nc.scalar.copy(v8[:, st, 512 * nhp : 512 * (nhp + 1)], psv[:])

            # ---- attention machinery, software-pipelined: scores issue
            # `la` items ahead of their exp/ctx/den so the PE never waits on
            # the ACT exp chain. ctx/den accumulation pairs adjacent sk
            # tiles into fp8 DoubleRow matmuls (K=256) where precision
            # allows; only tile (t=0, sh=0) -- the sole support of queries
            # with <128 valid keys -- stays bf16.
            # PLAN[sh]: per tile (in ALLOWED order): (t, (exkey, j, c0,
            # dtype), [(kind, tiles, lo, hi), ...]) where kind is
            # 'bf' (bf16 solo), 's8' (fp8 solo), 'dr' (fp8 DoubleRow pair).
            PLAN = {
                0: [
                    (0, ("A", None, 0, BF16), [("bf", (0,), 0, 512)]),
                    (1, ("P", 0, 128, FP8), [("s8", (1,), 128, 256)]),
                    (2, ("P", 1, 256, FP8), [("dr", (1, 2), 256, 512)]),
                    (3, ("B", None, 384, FP8), [("s8", (3,), 384, 512)]),
                ],
                1: [
                    (0, ("C", 0, 0, FP8), []),
                    (1, ("C", 1, 0, FP8), [("dr", (0, 1), 0, 512)]),
                    (2, ("D", 0, 0, FP8), []),
                    (3, ("D", 1, 0, FP8), [("dr", (2, 3), 0, 512)]),
                    (4, ("E", 0, 0, FP8), [("s8", (4,), 0, 128)]),
                    (5, ("E", 1, 128, FP8), [("dr", (4, 5), 128, 512)]),
                    (6, ("F", 0, 256, FP8), [("s8", (6,), 256, 384)]),
                    (7, ("F", 1, 384, FP8), [("dr", (6, 7), 384, 512)]),
                ],
            }

            class Attn:
                def __init__(self, heads, la):
                    self.work = []
                    for h in heads:
                        for sh in range(2):
                            plan = PLAN[sh]
                            for i, ent in enumerate(plan):
                                self.work.append((h, sh, i, ent, len(plan)))
                    self.la = la
                    self.scs = {}
                    self.extiles = {}
                    self.groups = {}
                    self.started = set()
                    self.nissued = 0
                    self.ndone = 0

                def issue_score(self):
                    j = self.nissued
                    h, sh, i, (t, (_k, _j, c0, _dt), _mms), _n = self.work[j]
                    sc = psp.tile([P, 512], F32, tag="ps")
                    nc.tensor.matmul(
                        sc[:, c0:512],
                        qkT[:, 8 + h, P * t : P * (t + 1)],
                        qkT[:, h, 512 * sh + c0 : 512 * (sh + 1)],
                        start=True,
                        stop=True,
                    )
                    self.scs[j] = sc
                    self.nissued += 1

                def exdst(self, gk, key, jj, dt):
                    tk = (gk, key)
                    if tk not in self.extiles:
                        nm = f"ex_{gk[0]}_{gk[1]}_{key}"
                        if jj is None:
                            self.extiles[tk] = exps.tile(
                                [P, 512], dt, tag="ex", name=nm
                            )
                        else:
                            self.extiles[tk] = exps.tile(
                                [P, 2, 512], dt, tag="ex2", name=nm
                            )
                    return self.extiles[tk]

                def advance(self, n):
                    for _ in range(n):
                        j = self.ndone
                        if j >= len(self.work):
                            return
                        while self.nissued < min(j + 1 + self.la, len(self.work)):
                            self.issue_score()
                        h, sh, i, (t, (key, jj, c0, dt), mms), ntiles = self.work[j]
                        gk = (h, sh)
                        sc = self.scs.pop(j)
                        ext = self.exdst(gk, key, jj, dt)
                        exsl = ext[:, c0:512] if jj is None else ext[:, jj, c0:512]
                        bias = (
                            bias_t[:, t - 4 : t - 3]
                            if (sh == 1 and t >= 4)
                            else 0.0
                        )
                        nc.scalar.activation(
                            exsl, sc[:, c0:512], EXP, bias=bias, scale=SCALE
                        )
                        if (t, sh) in PARTIAL:
                            cb = P * t - 512 * sh
                            msl = (
                                ext[:, cb : cb + P]
                                if jj is None
                                else ext[:, jj, cb : cb + P]
                            )
                            nc.vector.tensor_tensor(msl, msl, tri_t[:], op=MULT)
                        if gk not in self.groups:
                            ctx_ps = pscp.tile(
                                [P, 512], F32, tag="psc", name=f"ctxps_{h}_{sh}"
                            )
                            den_ps = psdp.tile(
                                [P, 512], F32, tag="psd", name=f"denps_{h}_{sh}"
                            )
                            self.groups[gk] = (ctx_ps, den_ps)
                        ctx_ps, den_ps = self.groups[gk]
                        for mi, (kind, tls, lo, hi) in enumerate(mms):
                            first = gk not in self.started
                            last = (i == ntiles - 1) and (mi == len(mms) - 1)
                            if kind == "bf":
                                vst = vsb[:, tls[0], DH * h : DH * (h + 1)]
                                ost = ones_t[:]
                                pm = None
                                mv = ext[:, lo:hi]
                            elif kind == "s8":
                                vst = v8[:, tls[0], DH * h : DH * (h + 1)]
                                ost = ones8_t[:, 0, :]
                                pm = None
                                mv = (
                                    ext[:, lo:hi]
                                    if jj is None
                                    else ext[:, jj, lo:hi]
                                )
                            else:  # dr pair
                                a = tls[0]
                                vst = v8[:, a : a + 2, DH * h : DH * (h + 1)]
                                ost = ones8_t[:, 0:2, :]
                                pm = DR
                                mv = ext[:, 0:2, lo:hi]
                            nc.tensor.matmul(
                                ctx_ps[:, lo:hi], vst, mv,
                                start=first, stop=last, perf_mode=pm,
                            )
                            nc.tensor.matmul(
                                den_ps[:, lo:hi], ost, mv,
                                start=first, stop=last, perf_mode=pm,
                            )
                            self.started.add(gk)
                        if i == ntiles - 1:
                            # 1/den via the fast custom-DVE reciprocal
                            # (~51 ULP, plenty for a softmax denominator);
                            # ctx_ps evacuates bf16 on DVE and the normalize
                            # multiply also runs on DVE.
                            rc = rcp.tile([P, 512], F32, tag="rc")
                            nc.vector.reciprocal_approx_fast(rc[:], den_ps[:])
                            cu = cup.tile([P, 512], BF16, tag="cu")
                            nc.vector.tensor_copy(cu[:], ctx_ps[:])
                            nc.vector.tensor_tensor(
                                ctxT[:, h, 512 * sh : 512 * (sh + 1)],
                                cu[:],
                                rc[:],
                                op=MULT,
                            )
                            del self.groups[gk]
                            self.started.discard(gk)
                            for k in [k for k in self.extiles if k[0] == gk]:
                                del self.extiles[k]
                        self.ndone += 1

            # v nhp0 first: heads 0..3 only need vsb columns 0..511.
            for st in range(8):
                emit_v_block(0, st)
            # interleave attention (h 0..3) with the nhp1 v blocks: the
            # attention inner loop is LDWEIGHTS-bound (3 stationaries per
            # item) while the v blocks are matmul-streaming-bound, so mixing
            # them keeps the PE dense.
            at1 = Attn(heads=range(4), la=2)
            for st in range(8):
                emit_v_block(1, st)
                at1.advance(12)
            at1.advance(len(at1.work))
            at2 = Attn(heads=range(4, 8), la=3)
            at2.advance(len(at2.work))

            # ---- output projection: outT[e, s] = sum_d wo[d, e] * ctxT[d, s]
            dmaq = [nc.sync, nc.scalar, nc.gpsimd]
            for me in range(16):
                for sh in range(2):
                    po = psp.tile([P, 512], F32, tag="ps")
                    for kd in range(8):
                        nc.tensor.matmul(
                            po[:],
                            wo_t[:, kd, P * me : P * (me + 1)],
                            ctxT[:, kd, 512 * sh : 512 * (sh + 1)],
                            start=(kd == 0),
                            stop=(kd == 7),
                        )
                    ot = otp.tile([P, 512], BF16, tag="ot")
                    if (me + sh) % 2 == 0:
                        nc.scalar.copy(ot[:], po[:])
                    else:
                        nc.vector.tensor_copy(ot[:], po[:])
                    eng = dmaq[(2 * me + sh) % 3]
                    eng.dma_start(
                        outT_d[:, me, 512 * sh : 512 * (sh + 1)], ot[:]
                    )
    nc.finalize()
    return nc


_NC_CACHE = None


def get_nc():
    global _NC_CACHE
    if _NC_CACHE is None:
        _NC_CACHE = build_nc()
    return _NC_CACHE


def _bf16(a):
    return np.ascontiguousarray(a.astype(ml_dtypes.bfloat16))


def _fp8(a):
    return np.ascontiguousarray(
        np.clip(a, -240.0, 240.0).astype(ml_dtypes.float8_e4m3)
    )


def make_in_maps(in_features, attention_mask, W_qkv, W_out):
    x = np.asarray(in_features, np.float32)
    am = np.asarray(attention_mask)
    Wqkv = np.asarray(W_qkv, np.float32)
    Wout = np.asarray(W_out, np.float32)
    seq_lens = am.astype(np.int64).sum(-1)

    perm = np.concatenate([np.arange(0, DH, 2), np.arange(1, DH, 2)])
    Wqh = Wqkv[0:D].reshape(H, DH, D)
    Wkh = Wqkv[D : 2 * D].reshape(H, DH, D)
    Wvh = Wqkv[2 * D : 3 * D].reshape(H, DH, D)

    half = DH // 2
    freq = THETA ** (-2.0 * np.arange(half, dtype=np.float64) / DH)
    ang = np.arange(S, dtype=np.float64)[:, None] * freq  # [S, 64]
    cosv = np.cos(ang).T.astype(np.float32)  # [64, S]
    sinv = np.sin(ang).T.astype(np.float32)
    cs = np.empty([P, 2, S], np.float32)
    cs[0:64, 0] = cosv
    cs[64:128, 0] = cosv
    cs[0:64, 1] = sinv
    cs[64:128, 1] = sinv
    cs = _bf16(cs)

    ones = np.ones([P, P], ml_dtypes.bfloat16)
    ones8 = np.ones([P, 2, P], ml_dtypes.float8_e4m3)
    pp = np.arange(P)[:, None]
    cc = np.arange(P)[None, :]
    tri = _bf16((pp <= cc).astype(np.float32))  # 1 on/above diag (sk<=sq)

    cvt_qk = _fp8 if FP8_QK else _bf16
    sw = SW if FP8_QK else 1.0
    sx = SX if FP8_QK else 1.0

    in_maps = []
    for c in range(8):
        b, g = c // 2, c % 2
        hs = slice(g * HPC, (g + 1) * HPC)
        # q/k weights: [p, mt, kt, m]; mt 0..7 q heads, 8..15 k heads
        wq = Wqh[hs][:, perm, :].reshape(DC, D) * sw
        wk = Wkh[hs][:, perm, :].reshape(DC, D) * sw
        wcat = np.concatenate([wq, wk], 0)               # [2048, 2048] (m, d)
        w8 = cvt_qk(wcat.reshape(16, P, 16, P).transpose(3, 0, 2, 1))
        # v weights: [p, nhp, kt, j]
        wv = Wvh[hs].reshape(DC, D)                      # [1024, 2048] (j, d)
        wv8 = _bf16(wv.reshape(2, 512, 16, P).transpose(3, 0, 2, 1))
        # out weights: [p, kd, e]
        wo8 = _bf16(
            Wout[:, g * DC : (g + 1) * DC].reshape(D, 8, P).transpose(2, 1, 0)
        )
        # x: [p, kt, s]
        xT = x[b].T                                      # [D, S]
        x8 = cvt_qk((xT * sx).reshape(16, P, S).transpose(1, 0, 2))
        xb = _bf16(xT.reshape(16, P, S).transpose(1, 0, 2))

        sl = int(seq_lens[b])
        bias = np.zeros([P, 4], np.float32)
        for t in range(4, 8):
            bias[:, t - 4] = np.where(t * P + np.arange(P) >= sl, NEG, 0.0)
        in_maps.append(
            dict(x8=x8, xb=xb, w8=w8, wv=wv8, wo=wo8, cs=cs,
                 tri=tri, bias=bias, ones=ones, ones8=ones8)
        )
    return in_maps


def kernel(in_features, past_k, past_v, attention_mask, W_qkv, W_out):
    nc = get_nc()
    in_maps = make_in_maps(in_features, attention_mask, W_qkv, W_out)
    res = run_bass_kernel_spmd(nc, in_maps, core_ids=list(range(8)))
    out = np.empty((B, S, D), np.float32)
    for b in range(B):
        acc = None
        for g in range(2):
            o = res.results[2 * b + g]["outT"].astype(np.float32)
            o = o.transpose(1, 0, 2).reshape(D, S)       # [e, s]
            acc = o if acc is None else acc + o
        out[b] = acc.T
    return out


# revision 25
# speedup vs baseline: 1.2978x; 1.2978x over previous
"""Trainium2 Bass kernel for fused attention prefill (nn_Attn_50740743635107).

Reference computation (fp32):
  qkv = x @ W_qkv.T ; split q,k,v ; interleaved RoPE on q,k ;
  scores = q k^T / sqrt(dh) with causal+valid_k mask ; softmax ;
  ctx = attn @ v ; out = ctx @ W_out.T

Shapes: B=4, S=1024, D=2048, H=16, DH=128.

Sharding: 8 cores = 4 batches x 2 head-groups (8 heads each).
Each core computes a partial out^T for its (batch, head-group);
the host sums the two head-group partials per batch and transposes.

v2 design (vs fp32r v1):
- All operands bf16 except the q/k projection which runs fp8e4m3 with
  DoubleRow perf mode (2 k-tiles per matmul). Host bakes power-of-2
  scales (x*8, Wqk*64); the combined descale folds into the softmax
  exp's free scale operand.
- Host pre-arranges every DRAM tensor into its exact SBUF layout so all
  DMAs are contiguous.
- RoPE in bf16 on DVE/GpSimd (4 tensor_tensor ops per 128-row tile, 2x
  DVE mode); q/k rows pre-permuted so even/odd pairs become contiguous
  halves.
- Causal mask applied multiplicatively AFTER exp (0/1 bf16 tri tile) so
  exp reads score PSUM directly; ragged seq_len masking stays as a
  per-partition additive bias inside the exp activation.
- Softmax reciprocal via ACT ln + exp(-x) (both functions live in the
  natural_log_exp table set) instead of the slow DVE iterative divide.
- Denominators via an all-ones stationary matmul accumulated alongside
  ctx, giving a partition-broadcast denominator for the normalize.
"""

from contextlib import ExitStack

import numpy as np
import ml_dtypes

import concourse.bass as bass
from concourse import bacc
from concourse import hw_specs
import concourse.mybir as mybir
import concourse.tile as tile
from concourse.bass_utils import run_bass_kernel_spmd

B, S, D, H = 4, 1024, 2048, 16
DH = 128           # head dim
HPC = 8            # heads per core
DC = HPC * DH      # 1024: d-range per core
P = 128
THETA = 10000.0
NEG = -60.0
F32 = mybir.dt.float32
BF16 = mybir.dt.bfloat16
FP8 = mybir.dt.float8e4
MULT = mybir.AluOpType.mult
ADD = mybir.AluOpType.add
SUB = mybir.AluOpType.subtract
EXP = mybir.ActivationFunctionType.Exp
LN = mybir.ActivationFunctionType.Ln
COPY = mybir.ActivationFunctionType.Copy
DR = mybir.MatmulPerfMode.DoubleRow

FP8_QK = True      # fp8 DoubleRow q/k projection (bf16 fallback if False)
SX, SW = 8.0, 64.0
_F = (SX * SW) if FP8_QK else 1.0
SCALE = float(1.0 / (_F * _F * np.sqrt(DH)))   # softmax exp input scale

# score tiles per head: ALLOWED[sq_half] = sk tiles; diagonal tiles are
# masked via the 0/1 tri tile, (t>=4, sh=1) also get the seq-len bias.
ALLOWED = {0: [0, 1, 2, 3], 1: [0, 1, 2, 3, 4, 5, 6, 7]}
PARTIAL = {(t, 0) for t in range(4)} | {(t, 1) for t in range(4, 8)}


def build_nc(reps=1):
    nc = bacc.Bacc()
    qk_dt = FP8 if FP8_QK else BF16
    x8_d = nc.dram_tensor("x8", [P, 16, S], qk_dt, kind="ExternalInput")
    xb_d = nc.dram_tensor("xb", [P, 16, P], BF16, kind="ExternalInput")
    w8_d = nc.dram_tensor("w8", [P, 16, 16, P], qk_dt, kind="ExternalInput")
    wv_d = nc.dram_tensor("wv", [P, 2, 16, 512], BF16, kind="ExternalInput")
    wv8_d = nc.dram_tensor("wv8", [P, 2, 16, 512], FP8, kind="ExternalInput")
    wo_d = nc.dram_tensor("wo", [P, 8, D], BF16, kind="ExternalInput")
    cs_d = nc.dram_tensor("cs", [P, 2, S], BF16, kind="ExternalInput")
    tri_d = nc.dram_tensor("tri", [P, P], BF16, kind="ExternalInput")
    bias_d = nc.dram_tensor("bias", [P, 4], F32, kind="ExternalInput")
    ones_d = nc.dram_tensor("ones", [P, P], BF16, kind="ExternalInput")
    ones8_d = nc.dram_tensor("ones8", [P, 2, P], FP8, kind="ExternalInput")
    outT_d = nc.dram_tensor("outT", [P, 16, S], BF16, kind="ExternalOutput")

    with tile.TileContext(nc) as tc:
      for rep in range(reps):
        with ExitStack() as es:
            pool = lambda **kw: es.enter_context(tc.tile_pool(**kw))
            qktp = pool(name="qkt", bufs=1)    # [128,16,1024] bf16 32K/p
            vsbp = pool(name="vsb", bufs=1)    # [128,8,1024] bf16 16K/p
            v8p = pool(name="v8", bufs=1)      # [128,8,1024] fp8 8K/p
            xbp = pool(name="xb", bufs=1)      # [128,16,128] bf16 4K/p
            x8p = pool(name="x8", bufs=1)      # fp8 16K/p (bf16 32K)
            wvp = pool(name="wv", bufs=2)      # [128,16,512] 16K/p bf16
            cstbp = pool(name="cstb", bufs=1)  # tri/bias/ones consts
            gtp = pool(name="gt", bufs=1)      # dma-gate scratch
            psp = pool(name="ps", bufs=4, space=bass.MemorySpace.PSUM)
            qkT = qktp.tile([P, 16, S], BF16, tag="qkt")
            vsb = vsbp.tile([P, 8, DC], BF16, tag="vsb")
            v8 = v8p.tile([P, 8, DC], FP8, tag="v8")
            xbt = xbp.tile([P, 16, P], BF16, tag="xb")
            x8t = x8p.tile([P, 16, S], qk_dt, tag="x8")
            tri_t = cstbp.tile([P, P], BF16, tag="tri")
            nc.gpsimd.dma_start(tri_t[:], tri_d[:])
            bias_t = cstbp.tile([P, 4], F32, tag="bias")
            nc.gpsimd.dma_start(bias_t[:], bias_d[:])
            ones_t = cstbp.tile([P, P], BF16, tag="ones")
            nc.gpsimd.dma_start(ones_t[:], ones_d[:])
            ones8_t = cstbp.tile([P, 2, P], FP8, tag="ones8")
            nc.gpsimd.dma_start(ones8_t[:], ones8_d[:])

            # ================= phase A1: q/k projection + RoPE ===============
            with ExitStack() as esA:
                poolA = lambda **kw: esA.enter_context(tc.tile_pool(**kw))
                wqkp = poolA(name="wqk", bufs=1)   # [128,16,16,128] 32K/p
                cstp = poolA(name="cst", bufs=1)   # cos/sin 4K/p
                ropep = poolA(name="rope", bufs=2) # [128,1024] bf16 2K/p
                pswp = poolA(name="psw", bufs=1, space=bass.MemorySpace.PSUM)
                w8t = wqkp.tile([P, 16, 16, P], qk_dt, tag="wqk")
                cs_t = cstp.tile([P, 2, S], BF16, tag="cs")
                with tc.high_priority():
                    # cs first (small) so PE warmup matmuls can start, then
                    # the tensors gating the first q/k m-tiles. x on sync,
                    # weights on the scalar HWDGE queue, in parallel.
                    nc.sync.dma_start(cs_t[:], cs_d[:])
                    nc.sync.dma_start(x8t[:, 0:8], x8_d[:, 0:8])
                    nc.scalar.dma_start(x8t[:, 8:16], x8_d[:, 8:16])
                    for quarter in range(4):
                        eng = nc.scalar if quarter % 2 == 0 else nc.sync
                        eng.dma_start(
                            w8t[:, 4 * quarter : 4 * (quarter + 1)],
                            w8_d[:, 4 * quarter : 4 * (quarter + 1)],
                        )
                    # PE warmup: ~14us of throwaway matmuls so the HAM clock
                    # gate reaches 8/8 and the x/w DMA wait is masked; gated
                    # only on the small cs transfer. Output is never read.
                    warm_ps = pswp.tile([P, 512], F32, tag="psw")
                    for w in range(56):
                        nc.tensor.matmul(
                            warm_ps[:], cs_t[:, 0, 0:P], cs_t[:, 0, 0:512],
                            start=True, stop=True,
                        )

                # ---- q/k projection (m-tile mt: 0..7 = q heads, 8..15 = k)
                for mt in range(16):
                    wqk = w8t[:, mt]
                    ps0 = psp.tile([P, 512], F32, tag="ps", name=f"qk{mt}_0")
                    ps1 = psp.tile([P, 512], F32, tag="ps", name=f"qk{mt}_1")
                    if FP8_QK:
                        for kk in range(8):
                            lw = wqk[:, 2 * kk : 2 * kk + 2, :]
                            nc.tensor.matmul(
                                ps0[:], lw, x8t[:, 2 * kk : 2 * kk + 2, 0:512],
                                start=(kk == 0), stop=(kk == 7), perf_mode=DR,
                            )
                            nc.tensor.matmul(
                                ps1[:], lw, x8t[:, 2 * kk : 2 * kk + 2, 512:1024],
                                start=(kk == 0), stop=(kk == 7), perf_mode=DR,
                            )
                    else:
                        for kt in range(16):
                            nc.tensor.matmul(
                                ps0[:], wqk[:, kt, :], x8t[:, kt, 0:512],
                                start=(kt == 0), stop=(kt == 15),
                            )
                            nc.tensor.matmul(
                                ps1[:], wqk[:, kt, :], x8t[:, kt, 512:1024],
                                start=(kt == 0), stop=(kt == 15),
                            )
                    col = qkT[:, mt, :]
                    nc.scalar.activation(col[:, 0:512], ps0[:], COPY)
                    nc.scalar.activation(col[:, 512:1024], ps1[:], COPY)
                    # ---- RoPE in place on qkT[:, mt, :].
                    # rows 0..63 = even dh (xe), 64..127 = odd dh (xo):
                    #   new_e = xe*cos - xo*sin ; new_o = xo*cos + xe*sin
                    # (both TT inputs must share a base partition; only the
                    # output AP may be partition-shifted)
                    tmp = ropep.tile([P, S], BF16, tag="rope")
                    nc.vector.tensor_tensor(
                        tmp[0:64, :], col[64:128, :], cs_t[64:128, 1, :], op=MULT
                    )
                    nc.vector.tensor_tensor(
                        tmp[64:128, :], col[0:64, :], cs_t[0:64, 1, :], op=MULT
                    )
                    nc.vector.tensor_tensor(col[:], col[:], cs_t[:, 0, :], op=MULT)
                    nc.vector.tensor_tensor(
                        col[0:64, :], col[0:64, :], tmp[0:64, :], op=SUB
                    )
                    nc.vector.tensor_tensor(
                        col[64:128, :], col[64:128, :], tmp[64:128, :], op=ADD
                    )

            # Pools for attention + out-projection, created only after the
            # A1 pools have released their SBUF (the allocator reserves pool
            # space in creation order for the pool's scope lifetime).
            wv8p = pool(name="wv8", bufs=2)    # [128,16,512] fp8 8K/p
            ctxp = pool(name="ctx", bufs=1)    # [128,8,1024] bf16 16K/p
            exps = pool(name="ex", bufs=10)    # [128,512] bf16 1K/p
            rcp = pool(name="rc", bufs=3)      # [128,512] f32 2K/p
            cup = pool(name="cu", bufs=3)      # [128,512] bf16 1K/p
            wop = pool(name="wo", bufs=1)      # [128,8,2048] bf16 32K/p
            otp = pool(name="ot", bufs=3)      # [128,512] bf16 1K/p
            pscp = pool(name="psc", bufs=2, space=bass.MemorySpace.PSUM)
            psdp = pool(name="psd", bufs=2, space=bass.MemorySpace.PSUM)

            # Late bulk DMAs (xb/wv/wo), gated on mt0 being roped so they do
            # not steal HBM bandwidth from the startup-critical x8/w8 loads.
            gate = gtp.tile([1, 2], BF16, tag="gt")
            nc.gpsimd.tensor_copy(gate[:], qkT[0:1, 0, 0:2])
            nc.gpsimd.dma_start(xbt[:], xb_d[:])
            wvts = []
            wv8ts = []
            for nhp in range(2):
                wvt = wvp.tile([P, 16, 512], BF16, tag="wv", name=f"wv{nhp}")
                nc.gpsimd.dma_start(wvt[:], wv_d[:, nhp])
                wvts.append(wvt)
                wv8t = wv8p.tile([P, 16, 512], FP8, tag="wv8", name=f"wv8{nhp}")
                nc.gpsimd.dma_start(wv8t[:], wv8_d[:, nhp])
                wv8ts.append(wv8t)
            ctxT = ctxp.tile([P, 8, S], BF16, tag="ctx")
            wo_t = wop.tile([P, 8, D], BF16, tag="wo")
            nc.gpsimd.dma_start(wo_t[:], wo_d[:])

            # ---- v projection block: v[s, j] = sum_d x[d, s] * Wv[j, d].
            # Tokens 0-127 (st=0) project in bf16 -- they are the whole
            # support of queries with <128 valid keys; later tokens only
            # feed softmax sums with >=129 terms, where fp8 projection
            # noise averages out, so st>=1 runs fp8 DoubleRow off x8/wv8
            # (scaled by SX*SW; the evacuation Copy descales).
            VDS = float(1.0 / _F)
            def emit_v_block(nhp, st):
                psv = psp.tile([P, 512], F32, tag="ps", name=f"v{nhp}_{st}")
                if st == 0 or not FP8_QK:
                    for kt in range(16):
                        nc.tensor.matmul(
                            psv[:],
                            xbt[:, kt, :] if FP8_QK
                            else x8t[:, kt, P * st : P * (st + 1)],
                            wvts[nhp][:, kt, :],
                            start=(kt == 0),
                            stop=(kt == 15),
                        )
                    sc_ = 1.0
                else:
                    for kk in range(8):
                        nc.tensor.matmul(
                            psv[:],
                            x8t[:, 2 * kk : 2 * kk + 2, P * st : P * (st + 1)],
                            wv8ts[nhp][:, 2 * kk : 2 * kk + 2, :],
                            start=(kk == 0),
                            stop=(kk == 7),
                            perf_mode=DR,
                        )
                    sc_ = VDS
                nc.scalar.activation(
                    vsb[:, st, 512 * nhp : 512 * (nhp + 1)], psv[:], COPY,
                    scale=sc_,
                )
                nc.scalar.activation(
                    v8[:, st, 512 * nhp : 512 * (nhp + 1)], psv[:], COPY,
                    scale=sc_,
                )

            # ---- attention machinery, software-pipelined: scores issue
            # `la` items ahead of their exp/ctx/den so the PE never waits on
            # the ACT exp chain. ctx/den accumulation pairs adjacent sk
            # tiles into fp8 DoubleRow matmuls (K=256) where precision
            # allows; only tile (t=0, sh=0) -- the sole support of queries
            # with <128 valid keys -- stays bf16.
            # PLAN[sh]: per tile (in ALLOWED order): (t, (exkey, j, c0,
            # dtype), [(kind, tiles, lo, hi), ...]) where kind is
            # 'bf' (bf16 solo), 's8' (fp8 solo), 'dr' (fp8 DoubleRow pair).
            PLAN = {
                0: [
                    (0, ("A", None, 0, BF16), [("bf", (0,), 0, 512)]),
                    (1, ("P", 0, 128, FP8), [("s8", (1,), 128, 256)]),
                    (2, ("P", 1, 256, FP8), [("dr", (1, 2), 256, 512)]),
                    (3, ("B", None, 384, FP8), [("s8", (3,), 384, 512)]),
                ],
                1: [
                    (0, ("C", 0, 0, FP8), []),
                    (1, ("C", 1, 0, FP8), [("dr", (0, 1), 0, 512)]),
                    (2, ("D", 0, 0, FP8), []),
                    (3, ("D", 1, 0, FP8), [("dr", (2, 3), 0, 512)]),
                    (4, ("E", 0, 0, FP8), [("s8", (4,), 0, 128)]),
                    (5, ("E", 1, 128, FP8), [("dr", (4, 5), 128, 512)]),
                    (6, ("F", 0, 256, FP8), [("s8", (6,), 256, 384)]),
                    (7, ("F", 1, 384, FP8), [("dr", (6, 7), 384, 512)]),
                ],
            }

            class Attn:
                def __init__(self, groups, la):
                    self.work = []
                    for h, sh in groups:
                        plan = PLAN[sh]
                        for i, ent in enumerate(plan):
                            self.work.append((h, sh, i, ent, len(plan)))
                    self.la = la
                    self.scs = {}
                    self.extiles = {}
                    self.groups = {}
                    self.started = set()
                    self.nissued = 0
                    self.ndone = 0

                def issue_score(self):
                    j = self.nissued
                    h, sh, i, (t, (_k, _j, c0, _dt), _mms), _n = self.work[j]
                    sc = psp.tile([P, 512], F32, tag="ps")
                    nc.tensor.matmul(
                        sc[:, c0:512],
                        qkT[:, 8 + h, P * t : P * (t + 1)],
                        qkT[:, h, 512 * sh + c0 : 512 * (sh + 1)],
                        start=True,
                        stop=True,
                    )
                    self.scs[j] = sc
                    self.nissued += 1

                def exdst(self, gk, key, jj, dt):
                    tk = (gk, key)
                    if tk not in self.extiles:
                        nm = f"ex_{gk[0]}_{gk[1]}_{key}"
                        if jj is None:
                            self.extiles[tk] = exps.tile(
                                [P, 512], dt, tag="ex", name=nm
                            )
                        else:
                            self.extiles[tk] = exps.tile(
                                [P, 2, 512], dt, tag="ex2", name=nm
                            )
                    return self.extiles[tk]

                def advance(self, n):
                    for _ in range(n):
                        j = self.ndone
                        if j >= len(self.work):
                            return
                        while self.nissued < min(j + 1 + self.la, len(self.work)):
                            self.issue_score()
                        h, sh, i, (t, (key, jj, c0, dt), mms), ntiles = self.work[j]
                        gk = (h, sh)
                        sc = self.scs.pop(j)
                        ext = self.exdst(gk, key, jj, dt)
                        exsl = ext[:, c0:512] if jj is None else ext[:, jj, c0:512]
                        bias = (
                            bias_t[:, t - 4 : t - 3]
                            if (sh == 1 and t >= 4)
                            else 0.0
                        )
                        nc.scalar.activation(
                            exsl, sc[:, c0:512], EXP, bias=bias, scale=SCALE
                        )
                        if (t, sh) in PARTIAL:
                            cb = P * t - 512 * sh
                            msl = (
                                ext[:, cb : cb + P]
                                if jj is None
                                else ext[:, jj, cb : cb + P]
                            )
                            nc.vector.tensor_tensor(msl, msl, tri_t[:], op=MULT)
                        if gk not in self.groups:
                            ctx_ps = pscp.tile(
                                [P, 512], F32, tag="psc", name=f"ctxps_{h}_{sh}"
                            )
                            den_ps = psdp.tile(
                                [P, 512], F32, tag="psd", name=f"denps_{h}_{sh}"
                            )
                            self.groups[gk] = (ctx_ps, den_ps)
                        ctx_ps, den_ps = self.groups[gk]
                        for mi, (kind, tls, lo, hi) in enumerate(mms):
                            first = gk not in self.started
                            last = (i == ntiles - 1) and (mi == len(mms) - 1)
                            if kind == "bf":
                                vst = vsb[:, tls[0], DH * h : DH * (h + 1)]
                                ost = ones_t[:]
                                pm = None
                                mv = ext[:, lo:hi]
                            elif kind == "s8":
                                vst = v8[:, tls[0], DH * h : DH * (h + 1)]
                                ost = ones8_t[:, 0, :]
                                pm = None
                                mv = (
                                    ext[:, lo:hi]
                                    if jj is None
                                    else ext[:, jj, lo:hi]
                                )
                            else:  # dr pair
                                a = tls[0]
                                vst = v8[:, a : a + 2, DH * h : DH * (h + 1)]
                                ost = ones8_t[:, 0:2, :]
                                pm = DR
                                mv = ext[:, 0:2, lo:hi]
                            nc.tensor.matmul(
                                ctx_ps[:, lo:hi], vst, mv,
                                start=first, stop=last, perf_mode=pm,
                            )
                            nc.tensor.matmul(
                                den_ps[:, lo:hi], ost, mv,
                                start=first, stop=last, perf_mode=pm,
                            )
                            self.started.add(gk)
                        if i == ntiles - 1:
                            # 1/den via the fast custom-DVE reciprocal
                            # (~51 ULP, plenty for a softmax denominator);
                            # ctx_ps evacuates bf16 on DVE and the normalize
                            # multiply also runs on DVE.
                            rc = rcp.tile([P, 512], F32, tag="rc")
                            nc.vector.reciprocal_approx_fast(rc[:], den_ps[:])
                            cu = cup.tile([P, 512], BF16, tag="cu")
                            nc.vector.tensor_copy(cu[:], ctx_ps[:])
                            nc.vector.tensor_tensor(
                                ctxT[:, h, 512 * sh : 512 * (sh + 1)],
                                cu[:],
                                rc[:],
                                op=MULT,
                            )
                            del self.groups[gk]
                            self.started.discard(gk)
                            for k in [k for k in self.extiles if k[0] == gk]:
                                del self.extiles[k]
                        self.ndone += 1

            # v nhp0 first: heads 0..3 only need vsb columns 0..511.
            for st in range(8):
                emit_v_block(0, st)
            # Interleave attention with the nhp1 v blocks and order the
            # groups so head>=4 groups (which need nhp1 v data) come late
            # enough: the attention inner loop is LDWEIGHTS- and ACT-bound
            # while the v blocks are matmul-streaming-bound, so mixing them
            # keeps the PE dense. (h, sh=1) for h>=4 needs every nhp1 tile.
            gseq = [(0, 0), (4, 0), (0, 1), (5, 0), (1, 0), (4, 1), (1, 1),
                    (6, 0), (2, 0), (5, 1), (2, 1), (7, 0), (3, 0), (6, 1),
                    (3, 1), (7, 1)]
            at = Attn(groups=gseq, la=2)
            sched = [("v", 1, 0), ("v", 1, 1), ("v", 1, 2), ("v", 1, 3),
                     ("g", 0, 0), ("v", 1, 4), ("g", 4, 0), ("v", 1, 5),
                     ("g", 0, 1), ("v", 1, 6), ("g", 5, 0), ("v", 1, 7)]
            for ent in sched:
                if ent[0] == "v":
                    emit_v_block(ent[1], ent[2])
                else:
                    _, h, sh = ent
                    at.advance(len(PLAN[sh]))
            at.advance(len(at.work))

            # ---- output projection: outT[e, s] = sum_d wo[d, e] * ctxT[d, s]
            dmaq = [nc.sync, nc.scalar, nc.gpsimd]
            for me in range(16):
                for sh in range(2):
                    po = psp.tile([P, 512], F32, tag="ps")
                    for kd in range(8):
                        nc.tensor.matmul(
                            po[:],
                            wo_t[:, kd, P * me : P * (me + 1)],
                            ctxT[:, kd, 512 * sh : 512 * (sh + 1)],
                            start=(kd == 0),
                            stop=(kd == 7),
                        )
                    ot = otp.tile([P, 512], BF16, tag="ot")
                    if (me + sh) % 2 == 0:
                        nc.scalar.copy(ot[:], po[:])
                    else:
                        nc.vector.tensor_copy(ot[:], po[:])
                    eng = dmaq[(2 * me + sh) % 3]
                    eng.dma_start(
                        outT_d[:, me, 512 * sh : 512 * (sh + 1)], ot[:]
                    )
    nc.finalize()
    return nc


_NC_CACHE = None


def get_nc():
    global _NC_CACHE
    if _NC_CACHE is None:
        _NC_CACHE = build_nc()
    return _NC_CACHE


def _bf16(a):
    return np.ascontiguousarray(a.astype(ml_dtypes.bfloat16))


def _fp8(a):
    return np.ascontiguousarray(
        np.clip(a, -240.0, 240.0).astype(ml_dtypes.float8_e4m3)
    )


def make_in_maps(in_features, attention_mask, W_qkv, W_out):
    x = np.asarray(in_features, np.float32)
    am = np.asarray(attention_mask)
    Wqkv = np.asarray(W_qkv, np.float32)
    Wout = np.asarray(W_out, np.float32)
    seq_lens = am.astype(np.int64).sum(-1)

    perm = np.concatenate([np.arange(0, DH, 2), np.arange(1, DH, 2)])
    Wqh = Wqkv[0:D].reshape(H, DH, D)
    Wkh = Wqkv[D : 2 * D].reshape(H, DH, D)
    Wvh = Wqkv[2 * D : 3 * D].reshape(H, DH, D)

    half = DH // 2
    freq = THETA ** (-2.0 * np.arange(half, dtype=np.float64) / DH)
    ang = np.arange(S, dtype=np.float64)[:, None] * freq  # [S, 64]
    cosv = np.cos(ang).T.astype(np.float32)  # [64, S]
    sinv = np.sin(ang).T.astype(np.float32)
    cs = np.empty([P, 2, S], np.float32)
    cs[0:64, 0] = cosv
    cs[64:128, 0] = cosv
    cs[0:64, 1] = sinv
    cs[64:128, 1] = sinv
    cs = _bf16(cs)

    ones = np.ones([P, P], ml_dtypes.bfloat16)
    ones8 = np.ones([P, 2, P], ml_dtypes.float8_e4m3)
    pp = np.arange(P)[:, None]
    cc = np.arange(P)[None, :]
    tri = _bf16((pp <= cc).astype(np.float32))  # 1 on/above diag (sk<=sq)

    cvt_qk = _fp8 if FP8_QK else _bf16
    sw = SW if FP8_QK else 1.0
    sx = SX if FP8_QK else 1.0

    in_maps = []
    for c in range(8):
        b, g = c // 2, c % 2
        hs = slice(g * HPC, (g + 1) * HPC)
        # q/k weights: [p, mt, kt, m]; mt 0..7 q heads, 8..15 k heads
        wq = Wqh[hs][:, perm, :].reshape(DC, D) * sw
        wk = Wkh[hs][:, perm, :].reshape(DC, D) * sw
        wcat = np.concatenate([wq, wk], 0)               # [2048, 2048] (m, d)
        w8 = cvt_qk(wcat.reshape(16, P, 16, P).transpose(3, 0, 2, 1))
        # v weights: [p, nhp, kt, j]
        wv = Wvh[hs].reshape(DC, D)                      # [1024, 2048] (j, d)
        wvb = _bf16(wv.reshape(2, 512, 16, P).transpose(3, 0, 2, 1))
        wv8 = cvt_qk((wv * sw).reshape(2, 512, 16, P).transpose(3, 0, 2, 1))
        # out weights: [p, kd, e]
        wo8 = _bf16(
            Wout[:, g * DC : (g + 1) * DC].reshape(D, 8, P).transpose(2, 1, 0)
        )
        # x: [p, kt, s]
        xT = x[b].T                                      # [D, S]
        x8 = cvt_qk((xT * sx).reshape(16, P, S).transpose(1, 0, 2))
        xb = _bf16(xT[:, 0:P].reshape(16, P, P).transpose(1, 0, 2))

        sl = int(seq_lens[b])
        bias = np.zeros([P, 4], np.float32)
        for t in range(4, 8):
            bias[:, t - 4] = np.where(t * P + np.arange(P) >= sl, NEG, 0.0)
        in_maps.append(
            dict(x8=x8, xb=xb, w8=w8, wv=wvb, wv8=wv8, wo=wo8, cs=cs,
                 tri=tri, bias=bias, ones=ones, ones8=ones8)
        )
    return in_maps


def kernel(in_features, past_k, past_v, attention_mask, W_qkv, W_out):
    nc = get_nc()
    in_maps = make_in_maps(in_features, attention_mask, W_qkv, W_out)
    res = run_bass_kernel_spmd(nc, in_maps, core_ids=list(range(8)))
    out = np.empty((B, S, D), np.float32)
    for b in range(B):
        acc = None
        for g in range(2):
            o = res.results[2 * b + g]["outT"].astype(np.float32)
            o = o.transpose(1, 0, 2).reshape(D, S)       # [e, s]
            acc = o if acc is None else acc + o
        out[b] = acc.T
    return out
